# revision 1
# baseline (speedup 1.0000x reference)
"""LSTM caption decoder on 8 TRN2 NeuronCores.

Problem: 24-step LSTMCell (B=128, E=512, H=1024) + vocab projection (V=12000).

Strategy (no collectives):
  - Every core computes the full-batch LSTM redundantly. B=128 exactly fills
    the PE array partition dim; sharding batch 16-way would run the array at
    12.5% utilization for the same wall time.
  - The FC vocab projection (2/3 of the FLOPs) is sharded column-wise:
    1500 vocab columns per core, interleaved into the timestep loop so the
    PE never idles waiting on the recurrent dependency chain.
  - Host does the embedding gather (pure indexing), the weight layout
    transposes, and the final vocab concat.

Layouts on device (per core):
  gates[t] = x_t @ W_ih.T + h_{t-1} @ W_hh.T + b   computed as
  psum[128B, 512-slice] accumulated over 4 xT chunks + 8 hT chunks (lhsT
  stationary = xT/hT [128K, 128M]), bias added in-place on PSUM by DVE,
  sigmoid/tanh on ACT reading PSUM, elementwise c/h on DVE, h -> hT via
  sbuf-to-sbuf DMA transpose, FC = 24 matmuls vs hT into 3x500 psum banks.

All matmul inputs bf16 (fp32 PSUM accumulation); c state fp32.
Measured end-to-end logits rel err vs fp32 reference: ~3.5e-3.
"""

import sys

if "/opt/trn_rl_repo" not in sys.path:
    sys.path.insert(0, "/opt/trn_rl_repo")

import numpy as np
import ml_dtypes

import concourse.bass as bass
import concourse.tile as tile
from concourse import mybir
from concourse.bass_utils import run_bass_kernel_spmd

B = 128
T = 24
E = 512
H = 1024
V = 12000
NCORES = 8
VS = V // NCORES          # 1500 vocab cols per core
KX = E // 128             # 4 contraction chunks for x-part
KH = H // 128             # 8 contraction chunks for h-part
NG = (4 * H) // 512       # 8 gate psum banks of 512
NVC = 3                   # vocab chunks per core (3 x 500)
VC = VS // NVC            # 500

F32 = mybir.dt.float32
BF16 = mybir.dt.bfloat16
AF = mybir.ActivationFunctionType

# gate bank n -> activation (torch order i,f,g,o over 4H)
_BANK_FUNC = [AF.Sigmoid] * 4 + [AF.Tanh] * 2 + [AF.Sigmoid] * 2


def _split_multi_waits(nc) -> int:
    """Walrus here allows exactly one sync-wait per 64B instruction (one
    NEURON_ISA_TPB_EVENTS slot). Tile attaches all outstanding waits to one
    instruction; rewrite  inst[wA,wB,wC] -> nop[wA]; nop[wB]; inst[wC]."""
    n = 0
    for fn in nc.m.functions:
        for bb in fn.blocks:
            insts = bb.instructions
            out = []
            changed = False
            for inst in insts:
                si = getattr(inst, "sync_info", None)
                waits = list(si.on_wait) if si is not None and si.on_wait else []
                if len(waits) > 1:
                    changed = True
                    for w in waits[:-1]:
                        nop = mybir.InstNoOp(
                            name=nc.get_next_instruction_name(),
                            sync_info=mybir.SyncInfo(on_wait=[w], on_update=[]),
                            bass_nofuse=True,
                            engine=inst.engine,
                        )
                        nc.register_instruction(nop, overwrite=True)
                        out.append(nop)
                        n += 1
                    inst.sync_info = mybir.SyncInfo(
                        on_wait=[waits[-1]], on_update=list(si.on_update or [])
                    )
                out.append(inst)
            if changed:
                insts.clear()
                insts.extend(out)
    return n


def build_nc():
    nc = bass.Bass("TRN2", target_bir_lowering=False, debug=False, num_devices=NCORES)

    wih_d = nc.dram_tensor("wih", [128, KX, 4 * H], BF16, kind="ExternalInput").ap()
    whh_d = nc.dram_tensor("whh", [128, KH, 4 * H], BF16, kind="ExternalInput").ap()
    wfc_d = nc.dram_tensor("wfc", [128, KH, VS], BF16, kind="ExternalInput").ap()
    xt_d = nc.dram_tensor("xt", [T, 128, KX, B], BF16, kind="ExternalInput").ap()
    bg_d = nc.dram_tensor("bg", [128, 4 * H], F32, kind="ExternalInput").ap()
    bfc_d = nc.dram_tensor("bfc", [128, VS], F32, kind="ExternalInput").ap()
    ht0_d = nc.dram_tensor("ht0", [128, KH, B], BF16, kind="ExternalInput").ap()
    c0_d = nc.dram_tensor("c0", [B, H], F32, kind="ExternalInput").ap()
    out_d = nc.dram_tensor("logits", [T, B, VS], F32, kind="ExternalOutput").ap()

    with tile.TileContext(nc) as tc:
        with (
            tc.tile_pool(name="weights", bufs=1) as wpool,
            tc.tile_pool(name="xin", bufs=3) as xpool,
            tc.tile_pool(name="gact", bufs=1) as gpool,
            tc.tile_pool(name="state", bufs=1) as spool,
            tc.tile_pool(name="tmp", bufs=1) as tpool,
            tc.tile_pool(name="hbuf", bufs=2) as hpool,
            tc.tile_pool(name="lout", bufs=4) as lpool,
            tc.tile_pool(name="pg", bufs=6, space="PSUM") as pgpool,
            tc.tile_pool(name="pf", bufs=2, space="PSUM") as pfpool,
        ):
            # Prologue loads, consumer-ordered so step 0/1 matmuls start
            # as early as possible: xt[0] is tiny, then wih in gate-bank
            # slices (step-0 bank n only needs slice n), then whh slices
            # (step 1), then biases and the FC weight.
            # Step 0 (h_prev = c_prev = 0) is computed on the host in fp32;
            # hT_0 / c_0 arrive as tiny inputs. This removes the step-0
            # tail from the DMA-bound startup window, and FC_0 (which only
            # needs hT_0 + wfc) becomes instant PE work while whh streams.
            wih = wpool.tile([128, KX, 4 * H], BF16)
            whh = wpool.tile([128, KH, 4 * H], BF16)
            bg = wpool.tile([128, 4 * H], F32)
            wfc = wpool.tile([128, KH, VS], BF16)
            bfc = wpool.tile([128, VS], F32)
            ht0 = hpool.tile([128, KH, B], BF16, tag="hT")
            nc.sync.dma_start(ht0[:], ht0_d[:])
            c = spool.tile([B, H], F32)
            nc.sync.dma_start(c[:], c0_d[:])
            xt1 = xpool.tile([128, KX, B], BF16, tag="xt")
            nc.sync.dma_start(xt1[:], xt_d[1])
            # wfc first: FC_0 is the only PE work with no other deps
            for v in range(NVC):
                vsl = slice(v * VC, (v + 1) * VC)
                nc.sync.dma_start(wfc[:, :, vsl], wfc_d[:, :, vsl])
            nc.sync.dma_start(bfc[:], bfc_d[:])
            for n in range(NG):
                nsl = slice(n * 512, (n + 1) * 512)
                nc.sync.dma_start(wih[:, :, nsl], wih_d[:, :, nsl])
                nc.sync.dma_start(bg[:, nsl], bg_d[:, nsl])

            hT_prev = ht0

            # gate-bank order: i(0,1), g(4,5), f(2,3), o(6,7) so the
            # c-chain (needs i,g then f) starts before o's sigmoid lands
            bank_order = [0, 1, 4, 5, 2, 3, 6, 7]

            def emit_fc(t, hT):
                # FC shard: logits[t] = h_t @ Wfc.T + bfc.
                # v-outer so only pf bufs=2 psum banks are live at once.
                for v in range(NVC):
                    fp = pfpool.tile([B, VC], F32, tag="pf")
                    for k in range(KH):
                        nc.tensor.matmul(
                            fp[:], hT[:, k, :], wfc[:, k, v * VC:(v + 1) * VC],
                            start=(k == 0), stop=(k == KH - 1),
                        )
                    lo = lpool.tile([B, VC], F32, tag="lo")
                    nc.vector.tensor_add(lo[:], fp[:], bfc[:, v * VC:(v + 1) * VC])
                    nc.scalar.dma_start(out_d[t, :, v * VC:(v + 1) * VC], lo[:])

            for t in range(1, T):
                if t == 1:
                    xt = xt1
                    # FC_0: instant PE work while whh streams in
                    emit_fc(0, ht0)
                    for n in bank_order:
                        nsl = slice(n * 512, (n + 1) * 512)
                        nc.sync.dma_start(whh[:, :, nsl], whh_d[:, :, nsl])
                else:
                    xt = xpool.tile([128, KX, B], BF16, tag="xt")
                    nc.sync.dma_start(xt[:], xt_d[t])

                # ---- gates: psum[n] = sum_k xT_k.T@Wih_k + hT_k.T@Whh_k ----
                gact = gpool.tile([B, 4 * H], F32, tag="gact")
                for n in bank_order:
                    ps = pgpool.tile([B, 512], F32, tag="pg")
                    nsl = slice(n * 512, (n + 1) * 512)
                    for k in range(KX):
                        nc.tensor.matmul(
                            ps[:], xt[:, k, :], wih[:, k, nsl],
                            start=(k == 0), stop=False,
                        )
                    for k in range(KH):
                        nc.tensor.matmul(
                            ps[:], hT_prev[:, k, :], whh[:, k, nsl],
                            start=False, stop=(k == KH - 1),
                        )
                    # bias in-place on psum, then activation PSUM -> SBUF
                    nc.vector.tensor_add(ps[:], ps[:], bg[:, nsl])
                    nc.scalar.activation(gact[:, nsl], ps[:], _BANK_FUNC[n])

                # FC of the PREVIOUS step: ready PE work that fills the
                # array while this step's activation/c/h/transpose tail
                # runs on DVE/ACT/DMA. (In-order PE stream: putting FC_t
                # here would stall the PE on the h_t transpose.)
                if t > 1:
                    emit_fc(t - 1, hT_prev)

                i_g = gact[:, 0:H]
                f_g = gact[:, H:2 * H]
                g_g = gact[:, 2 * H:3 * H]
                o_g = gact[:, 3 * H:4 * H]

                # ---- c, h ----
                ig = tpool.tile([B, H], F32, tag="ig")
                nc.vector.tensor_mul(ig[:], i_g, g_g)
                nc.vector.tensor_mul(c[:], c[:], f_g)
                nc.vector.tensor_add(c[:], c[:], ig[:])
                # ---- h, then h -> hT in halves so the first hT chunks land
                # early (transposes ride the scalar engine's DMA queue: the
                # sync queue carries the weight/x/logit streams and would
                # serialize them behind it).
                tanh_c = tpool.tile([B, H], F32, tag="tanh_c")
                h_bf = hpool.tile([B, H], BF16, tag="h_bf")
                hT = hpool.tile([128, KH, B], BF16, tag="hT")
                HH = H // 2
                for half in range(2):
                    hsl = slice(half * HH, (half + 1) * HH)
                    nc.scalar.activation(tanh_c[:, hsl], c[:, hsl], AF.Tanh)
                    nc.vector.tensor_mul(h_bf[:, hsl], o_g[:, hsl], tanh_c[:, hsl])
                    # one half per HWDGE engine so the two transposes run on
                    # different queues in parallel instead of serializing
                    eng = nc.scalar if half == 0 else nc.sync
                    eng.dma_start_transpose(
                        hT[:, half * (KH // 2):(half + 1) * (KH // 2), :],
                        h_bf[:, hsl])
                hT_prev = hT

            emit_fc(T - 1, hT_prev)

    _split_multi_waits(nc)
    return nc


_NC_CACHE = None


def _get_nc():
    global _NC_CACHE
    if _NC_CACHE is None:
        _NC_CACHE = build_nc()
    return _NC_CACHE


def _prep_inputs(encoder_output, captions, embed_table, W_ih, W_hh, b_ih, b_hh,
                 W_fc, b_fc):
    bf = ml_dtypes.bfloat16
    enc = np.asarray(encoder_output, np.float32)
    cap = np.asarray(captions).astype(np.int64)
    emb = np.asarray(embed_table, np.float32)
    W_ih = np.asarray(W_ih, np.float32)
    W_hh = np.asarray(W_hh, np.float32)
    W_fc = np.asarray(W_fc, np.float32)
    bg = (np.asarray(b_ih, np.float32) + np.asarray(b_hh, np.float32))
    b_fc = np.asarray(b_fc, np.float32)

    X = np.empty((T, B, E), np.float32)
    X[0] = enc
    X[1:] = emb[cap[:, : T - 1]].transpose(1, 0, 2)
    # xt[t,p,k,b] = X[t,b,k*128+p]
    xt = np.ascontiguousarray(
        X.reshape(T, B, KX, 128).transpose(0, 3, 2, 1)).astype(bf)

    # step 0 on host, fp32 (h_prev = c_prev = 0)
    gates0 = enc @ W_ih.T + bg
    i0, f0, g0, o0 = np.split(gates0, 4, axis=-1)
    sig = lambda z: 1.0 / (1.0 + np.exp(-z))
    c0 = sig(i0) * np.tanh(g0)
    h0 = sig(o0) * np.tanh(c0)
    ht0 = np.ascontiguousarray(
        h0.T.reshape(KH, 128, B).transpose(1, 0, 2)).astype(bf)
    wih = np.ascontiguousarray(
        W_ih.reshape(4 * H, KX, 128).transpose(2, 1, 0)).astype(bf)
    whh = np.ascontiguousarray(
        W_hh.reshape(4 * H, KH, 128).transpose(2, 1, 0)).astype(bf)
    bg_t = np.ascontiguousarray(np.broadcast_to(bg, (128, 4 * H)))

    common = {"wih": wih, "whh": whh, "xt": xt, "bg": bg_t,
              "ht0": ht0, "c0": np.ascontiguousarray(c0, np.float32)}
    in_maps = []
    for ci in range(NCORES):
        sl = slice(ci * VS, (ci + 1) * VS)
        wfc = np.ascontiguousarray(
            W_fc[sl].reshape(VS, KH, 128).transpose(2, 1, 0)).astype(bf)
        bfc = np.ascontiguousarray(np.broadcast_to(b_fc[sl], (128, VS)))
        in_maps.append({**common, "wfc": wfc, "bfc": bfc})
    return in_maps


def run_on_device(in_maps, trace=False, **kw):
    nc = _get_nc()
    return run_bass_kernel_spmd(
        nc, in_maps, list(range(NCORES)), trace=trace, **kw)


def kernel(encoder_output, captions, embed_table, W_ih, W_hh, b_ih, b_hh,
           W_fc, b_fc):
    in_maps = _prep_inputs(encoder_output, captions, embed_table,
                           W_ih, W_hh, b_ih, b_hh, W_fc, b_fc)
    res = run_on_device(in_maps)
    shards = [np.asarray(res.results[ci]["logits"]) for ci in range(NCORES)]
    full = np.concatenate(shards, axis=-1)  # [T, B, V]
    return np.ascontiguousarray(full.transpose(1, 0, 2))  # [B, T, V]



# revision 2
# speedup vs baseline: 1.0228x; 1.0228x over previous
"""LSTM caption decoder on 8 TRN2 NeuronCores — fp8 DoubleRow gates.

Problem: 24-step LSTMCell (B=128, E=512, H=1024) + vocab projection (V=12000).

Strategy (no collectives), evolved from the bf16 baseline:
  - Every core computes the full-batch LSTM redundantly; FC vocab projection
    sharded column-wise 8 ways (1500 cols/core), interleaved into the loop.
  - Gate matmuls run in fp8 e4m3 with MatmulPerfMode.DoubleRow: one
    instruction contracts TWO 128-row K-chunks (measured 2x bf16 MACs/s).
    Scaling keeps everything in e4m3's normal range: x*64, h*64, W*16;
    PSUM holds 1024*gates; ACT applies 1/1024 with the activation.
  - Gate bias folded into the matmul as a K=1 ones-row.
  - h -> hT via PE transposes (identity matmul) instead of DMA transpose:
    the 6us element-strided DMA transpose was the per-step critical path and
    its PE idle window dropped the clock to the mid p-state (427ns matmuls).
    The transpose PSUM tile feeds BOTH the bf16 hT (DVE copy, for FC) and
    the fp8 hT8 (ACT Copy scale=64, for the recurrence).
  - PE stream is software-pipelined so it never idles: per segment
    [bias+x(t) banks 0-3] [transposes(t-1)] [FC(t-1) + h-DR(t) + bias+x(t)
    interleaved] — x-part/bias have no recurrent dependency and cover the
    h(t-1) tail latency.

End-to-end rel err vs fp32 reference: 1.43e-2 (measured on HW; matches the
numpy sim of this exact scheme; W-quantization dominates).
"""

import sys

if "/opt/trn_rl_repo" not in sys.path:
    sys.path.insert(0, "/opt/trn_rl_repo")

import numpy as np
import ml_dtypes

import concourse.bass as bass
import concourse.tile as tile
from concourse import mybir
from concourse.bass_utils import run_bass_kernel_spmd

B = 128
T = 24
E = 512
H = 1024
V = 12000
NCORES = 8
VS = V // NCORES          # 1500 vocab cols per core
KX = E // 128             # 4 contraction chunks for x-part
KH = H // 128             # 8 contraction chunks for h-part
NG = (4 * H) // 512       # 8 gate psum banks of 512
NVC = 3                   # vocab chunks per core (3 x 500)
VC = VS // NVC            # 500

XS = 64.0                 # fp8 scale for x and h
WS = 16.0                 # fp8 scale for W_ih / W_hh
GS = XS * WS              # psum carries GS * gates

F32 = mybir.dt.float32
BF16 = mybir.dt.bfloat16
F8 = mybir.dt.float8e4
AF = mybir.ActivationFunctionType
DR = mybir.MatmulPerfMode.DoubleRow

# gate bank n -> activation (torch order i,f,g,o over 4H)
_BANK_FUNC = [AF.Sigmoid] * 4 + [AF.Tanh] * 2 + [AF.Sigmoid] * 2


def _split_multi_waits(nc) -> int:
    """Walrus here allows exactly one sync-wait per 64B instruction (one
    NEURON_ISA_TPB_EVENTS slot). Tile attaches all outstanding waits to one
    instruction; rewrite  inst[wA,wB,wC] -> nop[wA]; nop[wB]; inst[wC]."""
    n = 0
    for fn in nc.m.functions:
        for bb in fn.blocks:
            insts = bb.instructions
            out = []
            changed = False
            for inst in insts:
                si = getattr(inst, "sync_info", None)
                waits = list(si.on_wait) if si is not None and si.on_wait else []
                if len(waits) > 1:
                    changed = True
                    for w in waits[:-1]:
                        nop = mybir.InstNoOp(
                            name=nc.get_next_instruction_name(),
                            sync_info=mybir.SyncInfo(on_wait=[w], on_update=[]),
                            bass_nofuse=True,
                            engine=inst.engine,
                        )
                        nc.register_instruction(nop, overwrite=True)
                        out.append(nop)
                        n += 1
                    inst.sync_info = mybir.SyncInfo(
                        on_wait=[waits[-1]], on_update=list(si.on_update or [])
                    )
                out.append(inst)
            if changed:
                insts.clear()
                insts.extend(out)
    return n


def build_nc():
    nc = bass.Bass("TRN2", target_bir_lowering=False, debug=False, num_devices=NCORES)

    wih_d = nc.dram_tensor("wih", [128, KX, 4 * H], F8, kind="ExternalInput").ap()
    whh_d = nc.dram_tensor("whh", [128, KH, 4 * H], F8, kind="ExternalInput").ap()
    wfc_d = nc.dram_tensor("wfc", [128, KH, VS], BF16, kind="ExternalInput").ap()
    xt_d = nc.dram_tensor("xt", [T, 128, KX, B], F8, kind="ExternalInput").ap()
    bg_d = nc.dram_tensor("bg", [128, 4 * H], BF16, kind="ExternalInput").ap()
    bfc_d = nc.dram_tensor("bfc", [128, VS], F32, kind="ExternalInput").ap()
    ht0_d = nc.dram_tensor("ht0", [128, KH, B], BF16, kind="ExternalInput").ap()
    ht08_d = nc.dram_tensor("ht08", [128, KH, B], F8, kind="ExternalInput").ap()
    c0_d = nc.dram_tensor("c0", [B, H], F32, kind="ExternalInput").ap()
    one_d = nc.dram_tensor("one", [128, B], BF16, kind="ExternalInput").ap()
    iden_d = nc.dram_tensor("iden", [128, 128], BF16, kind="ExternalInput").ap()
    out_d = nc.dram_tensor("logits", [T, B, VS], F32, kind="ExternalOutput").ap()

    with tile.TileContext(nc) as tc:
        with (
            tc.tile_pool(name="weights", bufs=1) as wpool,
            tc.tile_pool(name="xin", bufs=1) as xpool,
            tc.tile_pool(name="gact", bufs=2) as gpool,
            tc.tile_pool(name="state", bufs=1) as spool,
            tc.tile_pool(name="tmp", bufs=1) as tpool,
            tc.tile_pool(name="hbuf", bufs=2) as hpool,
            tc.tile_pool(name="lout", bufs=4) as lpool,
            tc.tile_pool(name="pg", bufs=4, space="PSUM") as pgpool,
            tc.tile_pool(name="pf", bufs=2, space="PSUM") as pfpool,
            tc.tile_pool(name="pt", bufs=2, space="PSUM") as ptpool,
        ):
            # Prologue loads, consumer-ordered so early PE work starts asap.
            wih = wpool.tile([128, KX, 4 * H], F8)
            whh = wpool.tile([128, KH, 4 * H], F8)
            bg = wpool.tile([128, 4 * H], BF16)
            one = wpool.tile([128, B], BF16)
            iden = wpool.tile([128, 128], BF16)
            wfc = wpool.tile([128, KH, VS], BF16)
            bfc = wpool.tile([128, VS], F32)
            ht0 = hpool.tile([128, KH, B], BF16, tag="hT")
            ht08 = hpool.tile([128, KH, B], F8, tag="hT8")
            nc.sync.dma_start(one[:], one_d[:])
            nc.sync.dma_start(bg[:], bg_d[:])
            nc.sync.dma_start(iden[:], iden_d[:])
            nc.scalar.dma_start(ht0[:], ht0_d[:])
            nc.scalar.dma_start(ht08[:], ht08_d[:])
            c = spool.tile([B, H], F32)
            nc.scalar.dma_start(c[:], c0_d[:])
            # wih first (bias+x matmuls are the earliest PE work), then wfc
            # (FC_0), then whh in bank order.
            for n in range(NG):
                nsl = slice(n * 512, (n + 1) * 512)
                nc.sync.dma_start(wih[:, :, nsl], wih_d[:, :, nsl])
            xt_all = xpool.tile([128, T, KX, B], F8)
            for tt in range(1, T):
                nc.sync.dma_start(xt_all[:, tt, :, :], xt_d[tt])
            for v in range(NVC):
                vsl = slice(v * VC, (v + 1) * VC)
                nc.scalar.dma_start(wfc[:, :, vsl], wfc_d[:, :, vsl])
            nc.scalar.dma_start(bfc[:], bfc_d[:])

            hT_prev = ht0
            hT8_prev = ht08

            # gate-bank order: i(0,1), g(4,5), f(2,3), o(6,7) so the
            # c-chain (needs i,g then f) starts before o's sigmoid lands
            bank_order = [0, 1, 4, 5, 2, 3, 6, 7]

            psum_of = {}      # bank -> live psum tile (bias+x accumulated)

            def emit_bias_x(n, xt):
                ps = pgpool.tile([B, 512], F32, tag="pg")
                psum_of[n] = ps
                nsl = slice(n * 512, (n + 1) * 512)
                nc.tensor.matmul(ps[:], one[:], bg[:, nsl],
                                 start=True, stop=False)
                for k2 in range(KX // 2):
                    nc.tensor.matmul(
                        ps[:], xt[:, 2 * k2:2 * k2 + 2, :],
                        wih[:, 2 * k2:2 * k2 + 2, nsl],
                        start=False, stop=False, perf_mode=DR,
                    )

            def emit_h(n, hT8, gact):
                ps = psum_of.pop(n)
                nsl = slice(n * 512, (n + 1) * 512)
                for k2 in range(KH // 2):
                    nc.tensor.matmul(
                        ps[:], hT8[:, 2 * k2:2 * k2 + 2, :],
                        whh[:, 2 * k2:2 * k2 + 2, nsl],
                        start=False, stop=(k2 == KH // 2 - 1), perf_mode=DR,
                    )
                nc.scalar.activation(gact[:, nsl], ps[:], _BANK_FUNC[n],
                                     scale=1.0 / GS)

            def emit_fc_chunk(t, hT, v):
                fp = pfpool.tile([B, VC], F32, tag="pf")
                for k in range(KH):
                    nc.tensor.matmul(
                        fp[:], hT[:, k, :], wfc[:, k, v * VC:(v + 1) * VC],
                        start=(k == 0), stop=(k == KH - 1),
                    )
                lo = lpool.tile([B, VC], F32, tag="lo")
                nc.vector.tensor_add(lo[:], fp[:], bfc[:, v * VC:(v + 1) * VC])
                nc.scalar.dma_start(out_d[t, :, v * VC:(v + 1) * VC], lo[:])

            def emit_transposes(h_bf):
                hT = hpool.tile([128, KH, B], BF16, tag="hT")
                hT8 = hpool.tile([128, KH, B], F8, tag="hT8")
                for ch in range(KH):
                    pt = ptpool.tile([128, B], BF16, tag="pt")
                    nc.tensor.transpose(
                        pt[:], h_bf[:, ch * 128:(ch + 1) * 128], iden[:])
                    nc.vector.tensor_copy(hT[:, ch, :], pt[:])
                    nc.scalar.activation(hT8[:, ch, :], pt[:], AF.Copy,
                                         scale=XS)
                return hT, hT8

            h_bf_prev = None

            for t in range(1, T):
                xt = xt_all[:, t, :, :]
                bo = bank_order

                # ---- (A0) bias + x-part for banks 0-3 (no h dependency) ----
                for n in bo[:4]:
                    emit_bias_x(n, xt)

                # ---- (B) transposes of h(t-1) -> hT, hT8 ----
                if t > 1:
                    hT_prev, hT8_prev = emit_transposes(h_bf_prev)

                if t == 1:
                    for n in bank_order:
                        nsl = slice(n * 512, (n + 1) * 512)
                        nc.sync.dma_start(whh[:, :, nsl], whh_d[:, :, nsl])

                gact = gpool.tile([B, 4 * H], F32, tag="gact")

                # ---- (C/D interleave): FC(t-1), h-DR(t), bias+x rest ----
                emit_fc_chunk(t - 1, hT_prev, 0)
                emit_h(bo[0], hT8_prev, gact)
                emit_h(bo[1], hT8_prev, gact)
                emit_bias_x(bo[4], xt)
                emit_h(bo[2], hT8_prev, gact)
                emit_bias_x(bo[5], xt)
                emit_h(bo[3], hT8_prev, gact)
                emit_bias_x(bo[6], xt)
                emit_fc_chunk(t - 1, hT_prev, 1)
                emit_h(bo[4], hT8_prev, gact)
                emit_bias_x(bo[7], xt)
                emit_h(bo[5], hT8_prev, gact)
                emit_fc_chunk(t - 1, hT_prev, 2)
                emit_h(bo[6], hT8_prev, gact)
                emit_h(bo[7], hT8_prev, gact)

                i_g = gact[:, 0:H]
                f_g = gact[:, H:2 * H]
                g_g = gact[:, 2 * H:3 * H]
                o_g = gact[:, 3 * H:4 * H]

                # ---- (E) c, h tail in halves ----
                ig = tpool.tile([B, H], F32, tag="ig")
                tanh_c = tpool.tile([B, H], F32, tag="tanh_c")
                h_bf = hpool.tile([B, H], BF16, tag="h_bf")
                HHH = H // 2
                for half in range(2):
                    hsl = slice(half * HHH, (half + 1) * HHH)
                    nc.vector.tensor_mul(ig[:, hsl], i_g[:, hsl], g_g[:, hsl])
                    nc.vector.tensor_mul(c[:, hsl], c[:, hsl], f_g[:, hsl])
                    nc.vector.tensor_add(c[:, hsl], c[:, hsl], ig[:, hsl])
                    nc.scalar.activation(tanh_c[:, hsl], c[:, hsl], AF.Tanh)
                    nc.vector.tensor_mul(h_bf[:, hsl], o_g[:, hsl],
                                         tanh_c[:, hsl])
                h_bf_prev = h_bf

            # epilogue: transpose h(T-1), FC(T-1)
            hT_last, _ = emit_transposes(h_bf_prev)
            for v in range(NVC):
                emit_fc_chunk(T - 1, hT_last, v)

    _split_multi_waits(nc)
    return nc


_NC_CACHE = None


def _get_nc():
    global _NC_CACHE
    if _NC_CACHE is None:
        _NC_CACHE = build_nc()
    return _NC_CACHE


def _prep_inputs(encoder_output, captions, embed_table, W_ih, W_hh, b_ih, b_hh,
                 W_fc, b_fc):
    bf = ml_dtypes.bfloat16
    f8 = ml_dtypes.float8_e4m3fn
    enc = np.asarray(encoder_output, np.float32)
    cap = np.asarray(captions).astype(np.int64)
    emb = np.asarray(embed_table, np.float32)
    W_ih = np.asarray(W_ih, np.float32)
    W_hh = np.asarray(W_hh, np.float32)
    W_fc = np.asarray(W_fc, np.float32)
    bg = (np.asarray(b_ih, np.float32) + np.asarray(b_hh, np.float32))
    b_fc = np.asarray(b_fc, np.float32)

    X = np.empty((T, B, E), np.float32)
    X[0] = enc
    X[1:] = emb[cap[:, : T - 1]].transpose(1, 0, 2)
    # xt[t,p,k,b] = X[t,b,k*128+p] * XS  (fp8)
    xt = np.ascontiguousarray(
        (X * XS).reshape(T, B, KX, 128).transpose(0, 3, 2, 1)).astype(f8)

    # step 0 on host, fp32 (h_prev = c_prev = 0)
    gates0 = enc @ W_ih.T + bg
    i0, f0, g0, o0 = np.split(gates0, 4, axis=-1)
    sig = lambda z: 1.0 / (1.0 + np.exp(-z))
    c0 = sig(i0) * np.tanh(g0)
    h0 = sig(o0) * np.tanh(c0)
    h0_bf = h0.astype(bf).astype(np.float32)
    ht0 = np.ascontiguousarray(
        h0_bf.T.reshape(KH, 128, B).transpose(1, 0, 2)).astype(bf)
    ht08 = np.ascontiguousarray(
        (h0_bf.T * XS).reshape(KH, 128, B).transpose(1, 0, 2)).astype(f8)
    wih = np.ascontiguousarray(
        (W_ih.T * WS).reshape(KX, 128, 4 * H).transpose(1, 0, 2)).astype(f8)
    whh = np.ascontiguousarray(
        (W_hh.T * WS).reshape(KH, 128, 4 * H).transpose(1, 0, 2)).astype(f8)
    bg_row = np.zeros((128, 4 * H), bf)
    bg_row[0] = (bg * GS).astype(bf)
    one_row = np.zeros((128, B), bf)
    one_row[0] = 1.0
    iden = np.eye(128, dtype=bf)

    common = {"wih": wih, "whh": whh, "xt": xt, "bg": bg_row, "one": one_row,
              "iden": iden, "ht0": ht0, "ht08": ht08,
              "c0": np.ascontiguousarray(c0, np.float32)}
    in_maps = []
    for ci in range(NCORES):
        sl = slice(ci * VS, (ci + 1) * VS)
        wfc = np.ascontiguousarray(
            W_fc[sl].reshape(VS, KH, 128).transpose(2, 1, 0)).astype(bf)
        bfc = np.ascontiguousarray(np.broadcast_to(b_fc[sl], (128, VS)))
        in_maps.append({**common, "wfc": wfc, "bfc": bfc})
    return in_maps


def run_on_device(in_maps, trace=False, **kw):
    nc = _get_nc()
    return run_bass_kernel_spmd(
        nc, in_maps, list(range(NCORES)), trace=trace, **kw)


def kernel(encoder_output, captions, embed_table, W_ih, W_hh, b_ih, b_hh,
           W_fc, b_fc):
    in_maps = _prep_inputs(encoder_output, captions, embed_table,
                           W_ih, W_hh, b_ih, b_hh, W_fc, b_fc)
    res = run_on_device(in_maps)
    shards = [np.asarray(res.results[ci]["logits"]) for ci in range(NCORES)]
    full = np.concatenate(shards, axis=-1)  # [T, B, V]
    return np.ascontiguousarray(full.transpose(1, 0, 2))  # [B, T, V]


# revision 3
# speedup vs baseline: 1.0304x; 1.0074x over previous
"""LSTM caption decoder on 8 TRN2 NeuronCores — fp8 DoubleRow gates.

Problem: 24-step LSTMCell (B=128, E=512, H=1024) + vocab projection (V=12000).

Strategy (no collectives), evolved from the bf16 baseline:
  - Every core computes the full-batch LSTM redundantly; FC vocab projection
    sharded column-wise 8 ways (1500 cols/core), interleaved into the loop.
  - Gate matmuls run in fp8 e4m3 with MatmulPerfMode.DoubleRow: one
    instruction contracts TWO 128-row K-chunks (measured 2x bf16 MACs/s).
    Scaling keeps everything in e4m3's normal range: x*64, h*64, W*16;
    PSUM holds 1024*gates; ACT applies 1/1024 with the activation.
  - Gate bias folded into the matmul as a K=1 ones-row.
  - h -> hT via PE transposes (identity matmul) instead of DMA transpose:
    the 6us element-strided DMA transpose was the per-step critical path and
    its PE idle window dropped the clock to the mid p-state (427ns matmuls).
    The transpose PSUM tile feeds BOTH the bf16 hT (DVE copy, for FC) and
    the fp8 hT8 (ACT Copy scale=64, for the recurrence).
  - PE stream is software-pipelined so it never idles: per segment
    [bias+x(t) banks 0-3] [transposes(t-1)] [FC(t-1) + h-DR(t) + bias+x(t)
    interleaved] — x-part/bias have no recurrent dependency and cover the
    h(t-1) tail latency.

End-to-end rel err vs fp32 reference: 1.43e-2 (measured on HW; matches the
numpy sim of this exact scheme; W-quantization dominates).
"""

import sys

if "/opt/trn_rl_repo" not in sys.path:
    sys.path.insert(0, "/opt/trn_rl_repo")

import numpy as np
import ml_dtypes

import concourse.bass as bass
import concourse.tile as tile
from concourse import mybir
from concourse.bass_utils import run_bass_kernel_spmd

B = 128
T = 24
E = 512
H = 1024
V = 12000
NCORES = 8
VS = V // NCORES          # 1500 vocab cols per core
KX = E // 128             # 4 contraction chunks for x-part
KH = H // 128             # 8 contraction chunks for h-part
NG = (4 * H) // 512       # 8 gate psum banks of 512
NVC = 3                   # vocab chunks per core (3 x 500)
VC = VS // NVC            # 500

XS = 64.0                 # fp8 scale for x and h
WS = 16.0                 # fp8 scale for W_ih / W_hh
GS = XS * WS              # psum carries GS * gates

F32 = mybir.dt.float32
BF16 = mybir.dt.bfloat16
F8 = mybir.dt.float8e4
AF = mybir.ActivationFunctionType
DR = mybir.MatmulPerfMode.DoubleRow

# gate bank n -> activation (torch order i,f,g,o over 4H)
_BANK_FUNC = [AF.Sigmoid] * 4 + [AF.Tanh] * 2 + [AF.Sigmoid] * 2


def _split_multi_waits(nc) -> int:
    """Walrus here allows exactly one sync-wait per 64B instruction (one
    NEURON_ISA_TPB_EVENTS slot). Tile attaches all outstanding waits to one
    instruction; rewrite  inst[wA,wB,wC] -> nop[wA]; nop[wB]; inst[wC]."""
    n = 0
    for fn in nc.m.functions:
        for bb in fn.blocks:
            insts = bb.instructions
            out = []
            changed = False
            for inst in insts:
                si = getattr(inst, "sync_info", None)
                waits = list(si.on_wait) if si is not None and si.on_wait else []
                if len(waits) > 1:
                    changed = True
                    for w in waits[:-1]:
                        nop = mybir.InstNoOp(
                            name=nc.get_next_instruction_name(),
                            sync_info=mybir.SyncInfo(on_wait=[w], on_update=[]),
                            bass_nofuse=True,
                            engine=inst.engine,
                        )
                        nc.register_instruction(nop, overwrite=True)
                        out.append(nop)
                        n += 1
                    inst.sync_info = mybir.SyncInfo(
                        on_wait=[waits[-1]], on_update=list(si.on_update or [])
                    )
                out.append(inst)
            if changed:
                insts.clear()
                insts.extend(out)
    return n


def build_nc():
    nc = bass.Bass("TRN2", target_bir_lowering=False, debug=False, num_devices=NCORES)

    wih_d = nc.dram_tensor("wih", [128, KX, 4 * H], F8, kind="ExternalInput").ap()
    whh_d = nc.dram_tensor("whh", [128, KH, 4 * H], F8, kind="ExternalInput").ap()
    wfc_d = nc.dram_tensor("wfc", [128, KH, VS], BF16, kind="ExternalInput").ap()
    xt_d = nc.dram_tensor("xt", [T, 128, KX, B], F8, kind="ExternalInput").ap()
    bg_d = nc.dram_tensor("bg", [128, 4 * H], BF16, kind="ExternalInput").ap()
    bfc_d = nc.dram_tensor("bfc", [128, VS], F32, kind="ExternalInput").ap()
    ht0_d = nc.dram_tensor("ht0", [128, KH, B], BF16, kind="ExternalInput").ap()
    ht08_d = nc.dram_tensor("ht08", [128, KH, B], F8, kind="ExternalInput").ap()
    c0_d = nc.dram_tensor("c0", [B, H], F32, kind="ExternalInput").ap()
    one_d = nc.dram_tensor("one", [128, B], BF16, kind="ExternalInput").ap()
    iden_d = nc.dram_tensor("iden", [128, 128], BF16, kind="ExternalInput").ap()
    out_d = nc.dram_tensor("logits", [T, B, VS], F32, kind="ExternalOutput").ap()

    with tile.TileContext(nc) as tc:
        with (
            tc.tile_pool(name="weights", bufs=1) as wpool,
            tc.tile_pool(name="xin", bufs=1) as xpool,
            tc.tile_pool(name="gact", bufs=2) as gpool,
            tc.tile_pool(name="state", bufs=1) as spool,
            tc.tile_pool(name="tmp", bufs=1) as tpool,
            tc.tile_pool(name="hbuf", bufs=2) as hpool,
            tc.tile_pool(name="lout", bufs=4) as lpool,
            tc.tile_pool(name="pg", bufs=4, space="PSUM") as pgpool,
            tc.tile_pool(name="pf", bufs=2, space="PSUM") as pfpool,
            tc.tile_pool(name="pt", bufs=2, space="PSUM") as ptpool,
        ):
            # Prologue loads, consumer-ordered so early PE work starts asap.
            wih = wpool.tile([128, KX, 4 * H], F8)
            whh = wpool.tile([128, KH, 4 * H], F8)
            bg = wpool.tile([128, 4 * H], BF16)
            one = wpool.tile([128, B], BF16)
            iden = wpool.tile([128, 128], BF16)
            wfc = wpool.tile([128, KH, VS], BF16)
            bfc = wpool.tile([128, VS], F32)
            ht0 = hpool.tile([128, KH, B], BF16, tag="hT")
            ht08 = hpool.tile([128, KH, B], F8, tag="hT8")
            nc.sync.dma_start(one[:], one_d[:])
            nc.sync.dma_start(bg[:], bg_d[:])
            nc.sync.dma_start(iden[:], iden_d[:])
            nc.scalar.dma_start(ht0[:], ht0_d[:])
            nc.scalar.dma_start(ht08[:], ht08_d[:])
            c = spool.tile([B, H], F32)
            nc.scalar.dma_start(c[:], c0_d[:])
            # wih first (bias+x matmuls are the earliest PE work), then wfc
            # (FC_0), then whh in bank order.
            for n in range(NG):
                nsl = slice(n * 512, (n + 1) * 512)
                nc.sync.dma_start(wih[:, :, nsl], wih_d[:, :, nsl])
            xt_all = xpool.tile([128, T, KX, B], F8)
            for tt in range(1, T):
                nc.sync.dma_start(xt_all[:, tt, :, :], xt_d[tt])
            for v in range(NVC):
                vsl = slice(v * VC, (v + 1) * VC)
                nc.scalar.dma_start(wfc[:, :, vsl], wfc_d[:, :, vsl])
            nc.scalar.dma_start(bfc[:], bfc_d[:])

            hT_prev = ht0
            hT8_prev = ht08

            # gate-bank order: i(0,1), g(4,5), f(2,3), o(6,7) so the
            # c-chain (needs i,g then f) starts before o's sigmoid lands
            bank_order = [0, 1, 4, 5, 2, 3, 6, 7]

            psum_of = {}      # bank -> live psum tile (bias+x accumulated)

            def emit_bias_x(n, xt):
                ps = pgpool.tile([B, 512], F32, tag="pg")
                psum_of[n] = ps
                nsl = slice(n * 512, (n + 1) * 512)
                nc.tensor.matmul(ps[:], one[:], bg[:, nsl],
                                 start=True, stop=False)
                for k2 in range(KX // 2):
                    nc.tensor.matmul(
                        ps[:], xt[:, 2 * k2:2 * k2 + 2, :],
                        wih[:, 2 * k2:2 * k2 + 2, nsl],
                        start=False, stop=False, perf_mode=DR,
                    )

            def emit_h(n, hT8, gact):
                ps = psum_of.pop(n)
                nsl = slice(n * 512, (n + 1) * 512)
                for k2 in range(KH // 2):
                    nc.tensor.matmul(
                        ps[:], hT8[:, 2 * k2:2 * k2 + 2, :],
                        whh[:, 2 * k2:2 * k2 + 2, nsl],
                        start=False, stop=(k2 == KH // 2 - 1), perf_mode=DR,
                    )
                nc.scalar.activation(gact[:, nsl], ps[:], _BANK_FUNC[n],
                                     scale=1.0 / GS)

            def emit_fc_chunk(t, hT, v):
                fp = pfpool.tile([B, VC], F32, tag="pf")
                for k in range(KH):
                    nc.tensor.matmul(
                        fp[:], hT[:, k, :], wfc[:, k, v * VC:(v + 1) * VC],
                        start=(k == 0), stop=(k == KH - 1),
                    )
                lo = lpool.tile([B, VC], F32, tag="lo")
                nc.vector.tensor_add(lo[:], fp[:], bfc[:, v * VC:(v + 1) * VC])
                nc.scalar.dma_start(out_d[t, :, v * VC:(v + 1) * VC], lo[:])

            def emit_transposes(h_bf):
                hT = hpool.tile([128, KH, B], BF16, tag="hT")
                hT8 = hpool.tile([128, KH, B], F8, tag="hT8")
                for ch in range(KH):
                    pt = ptpool.tile([128, B], BF16, tag="pt")
                    nc.tensor.transpose(
                        pt[:], h_bf[:, ch * 128:(ch + 1) * 128], iden[:])
                    nc.vector.tensor_copy(hT[:, ch, :], pt[:])
                    nc.scalar.activation(hT8[:, ch, :], pt[:], AF.Copy,
                                         scale=XS)
                return hT, hT8

            h_bf_prev = None

            for t in range(1, T):
                xt = xt_all[:, t, :, :]
                bo = bank_order

                # ---- (A0) bias + x-part for banks 0-3 (no h dependency) ----
                for n in bo[:4]:
                    emit_bias_x(n, xt)

                # ---- (B) transposes of h(t-1) -> hT, hT8 ----
                if t > 1:
                    hT_prev, hT8_prev = emit_transposes(h_bf_prev)

                if t == 1:
                    for n in bank_order:
                        nsl = slice(n * 512, (n + 1) * 512)
                        nc.sync.dma_start(whh[:, :, nsl], whh_d[:, :, nsl])

                gact = gpool.tile([B, 4 * H], F32, tag="gact")

                # ---- (C/D interleave): FC(t-1), h-DR(t), bias+x rest ----
                emit_fc_chunk(t - 1, hT_prev, 0)
                emit_h(bo[0], hT8_prev, gact)
                emit_h(bo[1], hT8_prev, gact)
                emit_bias_x(bo[4], xt)
                emit_h(bo[2], hT8_prev, gact)
                emit_bias_x(bo[5], xt)
                emit_h(bo[3], hT8_prev, gact)
                emit_bias_x(bo[6], xt)
                emit_fc_chunk(t - 1, hT_prev, 1)
                emit_h(bo[4], hT8_prev, gact)
                emit_bias_x(bo[7], xt)
                emit_h(bo[5], hT8_prev, gact)
                emit_fc_chunk(t - 1, hT_prev, 2)
                emit_h(bo[6], hT8_prev, gact)
                emit_h(bo[7], hT8_prev, gact)

                i_g = gact[:, 0:H]
                f_g = gact[:, H:2 * H]
                g_g = gact[:, 2 * H:3 * H]
                o_g = gact[:, 3 * H:4 * H]

                # ---- (E) c, h tail in halves ----
                ig = tpool.tile([B, H], F32, tag="ig")
                tanh_c = tpool.tile([B, H], F32, tag="tanh_c")
                h_bf = hpool.tile([B, H], BF16, tag="h_bf")
                HHH = H // 2
                for half in range(2):
                    hsl = slice(half * HHH, (half + 1) * HHH)
                    nc.vector.tensor_mul(ig[:, hsl], i_g[:, hsl], g_g[:, hsl])
                    nc.vector.tensor_mul(c[:, hsl], c[:, hsl], f_g[:, hsl])
                    nc.vector.tensor_add(c[:, hsl], c[:, hsl], ig[:, hsl])
                    nc.scalar.activation(tanh_c[:, hsl], c[:, hsl], AF.Tanh)
                    nc.vector.tensor_mul(h_bf[:, hsl], o_g[:, hsl],
                                         tanh_c[:, hsl])
                h_bf_prev = h_bf

            # epilogue: transpose h(T-1) halves interleaved with FC(T-1)
            hT_last = hpool.tile([128, KH, B], BF16, tag="hT")
            for ch in range(KH // 2):
                pt = ptpool.tile([128, B], BF16, tag="pt")
                nc.tensor.transpose(
                    pt[:], h_bf_prev[:, ch * 128:(ch + 1) * 128], iden[:])
                nc.vector.tensor_copy(hT_last[:, ch, :], pt[:])
            fp0 = pfpool.tile([B, VC], F32, tag="pf", name="fpe0")
            fp1 = pfpool.tile([B, VC], F32, tag="pf", name="fpe1")
            for v, fp in ((0, fp0), (1, fp1)):
                for k in range(KH // 2):
                    nc.tensor.matmul(
                        fp[:], hT_last[:, k, :],
                        wfc[:, k, v * VC:(v + 1) * VC],
                        start=(k == 0), stop=False)
            for ch in range(KH // 2, KH):
                pt = ptpool.tile([128, B], BF16, tag="pt")
                nc.tensor.transpose(
                    pt[:], h_bf_prev[:, ch * 128:(ch + 1) * 128], iden[:])
                nc.vector.tensor_copy(hT_last[:, ch, :], pt[:])
            for v, fp in ((0, fp0), (1, fp1)):
                for k in range(KH // 2, KH):
                    nc.tensor.matmul(
                        fp[:], hT_last[:, k, :],
                        wfc[:, k, v * VC:(v + 1) * VC],
                        start=False, stop=(k == KH - 1))
                lo = lpool.tile([B, VC], F32, tag="lo")
                nc.vector.tensor_add(lo[:], fp[:],
                                     bfc[:, v * VC:(v + 1) * VC])
                nc.scalar.dma_start(out_d[T - 1, :, v * VC:(v + 1) * VC], lo[:])
            emit_fc_chunk(T - 1, hT_last, 2)

    _split_multi_waits(nc)
    return nc


_NC_CACHE = None


def _get_nc():
    global _NC_CACHE
    if _NC_CACHE is None:
        _NC_CACHE = build_nc()
    return _NC_CACHE


def _prep_inputs(encoder_output, captions, embed_table, W_ih, W_hh, b_ih, b_hh,
                 W_fc, b_fc):
    bf = ml_dtypes.bfloat16
    f8 = ml_dtypes.float8_e4m3fn
    enc = np.asarray(encoder_output, np.float32)
    cap = np.asarray(captions).astype(np.int64)
    emb = np.asarray(embed_table, np.float32)
    W_ih = np.asarray(W_ih, np.float32)
    W_hh = np.asarray(W_hh, np.float32)
    W_fc = np.asarray(W_fc, np.float32)
    bg = (np.asarray(b_ih, np.float32) + np.asarray(b_hh, np.float32))
    b_fc = np.asarray(b_fc, np.float32)

    X = np.empty((T, B, E), np.float32)
    X[0] = enc
    X[1:] = emb[cap[:, : T - 1]].transpose(1, 0, 2)
    # xt[t,p,k,b] = X[t,b,k*128+p] * XS  (fp8)
    xt = np.ascontiguousarray(
        (X * XS).reshape(T, B, KX, 128).transpose(0, 3, 2, 1)).astype(f8)

    # step 0 on host, fp32 (h_prev = c_prev = 0)
    gates0 = enc @ W_ih.T + bg
    i0, f0, g0, o0 = np.split(gates0, 4, axis=-1)
    sig = lambda z: 1.0 / (1.0 + np.exp(-z))
    c0 = sig(i0) * np.tanh(g0)
    h0 = sig(o0) * np.tanh(c0)
    h0_bf = h0.astype(bf).astype(np.float32)
    ht0 = np.ascontiguousarray(
        h0_bf.T.reshape(KH, 128, B).transpose(1, 0, 2)).astype(bf)
    ht08 = np.ascontiguousarray(
        (h0_bf.T * XS).reshape(KH, 128, B).transpose(1, 0, 2)).astype(f8)
    wih = np.ascontiguousarray(
        (W_ih.T * WS).reshape(KX, 128, 4 * H).transpose(1, 0, 2)).astype(f8)
    whh = np.ascontiguousarray(
        (W_hh.T * WS).reshape(KH, 128, 4 * H).transpose(1, 0, 2)).astype(f8)
    bg_row = np.zeros((128, 4 * H), bf)
    bg_row[0] = (bg * GS).astype(bf)
    one_row = np.zeros((128, B), bf)
    one_row[0] = 1.0
    iden = np.eye(128, dtype=bf)

    common = {"wih": wih, "whh": whh, "xt": xt, "bg": bg_row, "one": one_row,
              "iden": iden, "ht0": ht0, "ht08": ht08,
              "c0": np.ascontiguousarray(c0, np.float32)}
    in_maps = []
    for ci in range(NCORES):
        sl = slice(ci * VS, (ci + 1) * VS)
        wfc = np.ascontiguousarray(
            W_fc[sl].reshape(VS, KH, 128).transpose(2, 1, 0)).astype(bf)
        bfc = np.ascontiguousarray(np.broadcast_to(b_fc[sl], (128, VS)))
        in_maps.append({**common, "wfc": wfc, "bfc": bfc})
    return in_maps


def run_on_device(in_maps, trace=False, **kw):
    nc = _get_nc()
    return run_bass_kernel_spmd(
        nc, in_maps, list(range(NCORES)), trace=trace, **kw)


def kernel(encoder_output, captions, embed_table, W_ih, W_hh, b_ih, b_hh,
           W_fc, b_fc):
    in_maps = _prep_inputs(encoder_output, captions, embed_table,
                           W_ih, W_hh, b_ih, b_hh, W_fc, b_fc)
    res = run_on_device(in_maps)
    shards = [np.asarray(res.results[ci]["logits"]) for ci in range(NCORES)]
    full = np.concatenate(shards, axis=-1)  # [T, B, V]
    return np.ascontiguousarray(full.transpose(1, 0, 2))  # [B, T, V]


# revision 4
# speedup vs baseline: 1.0317x; 1.0013x over previous
"""LSTM caption decoder on 8 TRN2 NeuronCores — fp8 DoubleRow gates.

Problem: 24-step LSTMCell (B=128, E=512, H=1024) + vocab projection (V=12000).

Strategy (no collectives), evolved from the bf16 baseline:
  - Every core computes the full-batch LSTM redundantly; FC vocab projection
    sharded column-wise 8 ways (1500 cols/core), interleaved into the loop.
  - Gate matmuls run in fp8 e4m3 with MatmulPerfMode.DoubleRow: one
    instruction contracts TWO 128-row K-chunks (measured 2x bf16 MACs/s).
    Scaling keeps everything in e4m3's normal range: x*64, h*64, W*16;
    PSUM holds 1024*gates; ACT applies 1/1024 with the activation.
  - Gate bias folded into the matmul as a K=1 ones-row.
  - h -> hT via PE transposes (identity matmul) instead of DMA transpose:
    the 6us element-strided DMA transpose was the per-step critical path and
    its PE idle window dropped the clock to the mid p-state (427ns matmuls).
    The transpose PSUM tile feeds BOTH the bf16 hT (DVE copy, for FC) and
    the fp8 hT8 (ACT Copy scale=64, for the recurrence).
  - PE stream is software-pipelined so it never idles: per segment
    [bias+x(t) banks 0-3] [transposes(t-1)] [FC(t-1) + h-DR(t) + bias+x(t)
    interleaved] — x-part/bias have no recurrent dependency and cover the
    h(t-1) tail latency.

End-to-end rel err vs fp32 reference: 1.43e-2 (measured on HW; matches the
numpy sim of this exact scheme; W-quantization dominates).
"""

import sys

if "/opt/trn_rl_repo" not in sys.path:
    sys.path.insert(0, "/opt/trn_rl_repo")

import numpy as np
import ml_dtypes

import concourse.bass as bass
import concourse.tile as tile
from concourse import mybir
from concourse.bass_utils import run_bass_kernel_spmd

B = 128
T = 24
E = 512
H = 1024
V = 12000
NCORES = 8
VS = V // NCORES          # 1500 vocab cols per core
KX = E // 128             # 4 contraction chunks for x-part
KH = H // 128             # 8 contraction chunks for h-part
NG = (4 * H) // 512       # 8 gate psum banks of 512
NVC = 3                   # vocab chunks per core (3 x 500)
VC = VS // NVC            # 500

XS = 64.0                 # fp8 scale for x and h
WS = 16.0                 # fp8 scale for W_ih / W_hh
GS = XS * WS              # psum carries GS * gates

F32 = mybir.dt.float32
BF16 = mybir.dt.bfloat16
F8 = mybir.dt.float8e4
AF = mybir.ActivationFunctionType
DR = mybir.MatmulPerfMode.DoubleRow

# gate bank n -> activation (torch order i,f,g,o over 4H)
_BANK_FUNC = [AF.Sigmoid] * 4 + [AF.Tanh] * 2 + [AF.Sigmoid] * 2


def _split_multi_waits(nc) -> int:
    """Walrus here allows exactly one sync-wait per 64B instruction (one
    NEURON_ISA_TPB_EVENTS slot). Tile attaches all outstanding waits to one
    instruction; rewrite  inst[wA,wB,wC] -> nop[wA]; nop[wB]; inst[wC]."""
    n = 0
    for fn in nc.m.functions:
        for bb in fn.blocks:
            insts = bb.instructions
            out = []
            changed = False
            for inst in insts:
                si = getattr(inst, "sync_info", None)
                waits = list(si.on_wait) if si is not None and si.on_wait else []
                if len(waits) > 1:
                    changed = True
                    for w in waits[:-1]:
                        nop = mybir.InstNoOp(
                            name=nc.get_next_instruction_name(),
                            sync_info=mybir.SyncInfo(on_wait=[w], on_update=[]),
                            bass_nofuse=True,
                            engine=inst.engine,
                        )
                        nc.register_instruction(nop, overwrite=True)
                        out.append(nop)
                        n += 1
                    inst.sync_info = mybir.SyncInfo(
                        on_wait=[waits[-1]], on_update=list(si.on_update or [])
                    )
                out.append(inst)
            if changed:
                insts.clear()
                insts.extend(out)
    return n


def build_nc():
    nc = bass.Bass("TRN2", target_bir_lowering=False, debug=False, num_devices=NCORES)

    wih_d = nc.dram_tensor("wih", [128, KX, 4 * H], F8, kind="ExternalInput").ap()
    whh_d = nc.dram_tensor("whh", [128, KH, 4 * H], F8, kind="ExternalInput").ap()
    wfc_d = nc.dram_tensor("wfc", [128, KH, VS], BF16, kind="ExternalInput").ap()
    xt_d = nc.dram_tensor("xt", [T, 128, KX, B], F8, kind="ExternalInput").ap()
    bg_d = nc.dram_tensor("bg", [1, 4 * H], BF16, kind="ExternalInput").ap()
    bfc_d = nc.dram_tensor("bfc", [128, VS], F32, kind="ExternalInput").ap()
    ht0_d = nc.dram_tensor("ht0", [128, KH, B], BF16, kind="ExternalInput").ap()
    ht08_d = nc.dram_tensor("ht08", [128, KH, B], F8, kind="ExternalInput").ap()
    c0_d = nc.dram_tensor("c0", [B, H], F32, kind="ExternalInput").ap()
    one_d = nc.dram_tensor("one", [128, B], BF16, kind="ExternalInput").ap()
    iden_d = nc.dram_tensor("iden", [128, 128], BF16, kind="ExternalInput").ap()
    out_d = nc.dram_tensor("logits", [T, B, VS], F32, kind="ExternalOutput").ap()

    with tile.TileContext(nc) as tc:
        with (
            tc.tile_pool(name="weights", bufs=1) as wpool,
            tc.tile_pool(name="xin", bufs=1) as xpool,
            tc.tile_pool(name="gact", bufs=2) as gpool,
            tc.tile_pool(name="state", bufs=1) as spool,
            tc.tile_pool(name="tmp", bufs=1) as tpool,
            tc.tile_pool(name="hbuf", bufs=2) as hpool,
            tc.tile_pool(name="lout", bufs=4) as lpool,
            tc.tile_pool(name="pg", bufs=4, space="PSUM") as pgpool,
            tc.tile_pool(name="pf", bufs=2, space="PSUM") as pfpool,
            tc.tile_pool(name="pt", bufs=2, space="PSUM") as ptpool,
        ):
            # Prologue loads, consumer-ordered so early PE work starts asap.
            wih = wpool.tile([128, KX, 4 * H], F8)
            whh = wpool.tile([128, KH, 4 * H], F8)
            bg = wpool.tile([128, 4 * H], BF16)
            one = wpool.tile([128, B], BF16)
            iden = wpool.tile([128, 128], BF16)
            wfc = wpool.tile([128, KH, VS], BF16)
            bfc = wpool.tile([128, VS], F32)
            ht0 = hpool.tile([128, KH, B], BF16, tag="hT")
            ht08 = hpool.tile([128, KH, B], F8, tag="hT8")
            nc.sync.dma_start(one[:], one_d[:])
            nc.sync.dma_start(bg[:], bg_d[:])
            nc.sync.dma_start(iden[:], iden_d[:])
            nc.scalar.dma_start(ht0[:], ht0_d[:])
            nc.scalar.dma_start(ht08[:], ht08_d[:])
            c = spool.tile([B, H], F32)
            nc.scalar.dma_start(c[:], c0_d[:])
            # wih first (bias+x matmuls are the earliest PE work), then wfc
            # (FC_0), then whh in bank order.
            for n in range(NG):
                nsl = slice(n * 512, (n + 1) * 512)
                nc.sync.dma_start(wih[:, :, nsl], wih_d[:, :, nsl])
            xt_all = xpool.tile([128, T, KX, B], F8)
            for tt in range(1, T):
                nc.sync.dma_start(xt_all[:, tt, :, :], xt_d[tt])
            for v in range(NVC):
                vsl = slice(v * VC, (v + 1) * VC)
                nc.scalar.dma_start(wfc[:, :, vsl], wfc_d[:, :, vsl])
            nc.scalar.dma_start(bfc[:], bfc_d[:])

            hT_prev = ht0
            hT8_prev = ht08

            # gate-bank order: i(0,1), g(4,5), f(2,3), o(6,7) so the
            # c-chain (needs i,g then f) starts before o's sigmoid lands
            bank_order = [0, 1, 4, 5, 2, 3, 6, 7]

            psum_of = {}      # bank -> live psum tile (bias+x accumulated)

            def emit_bias_x(n, xt):
                ps = pgpool.tile([B, 512], F32, tag="pg")
                psum_of[n] = ps
                nsl = slice(n * 512, (n + 1) * 512)
                nc.tensor.matmul(ps[:], one[:], bg[:, nsl],
                                 start=True, stop=False)
                for k2 in range(KX // 2):
                    nc.tensor.matmul(
                        ps[:], xt[:, 2 * k2:2 * k2 + 2, :],
                        wih[:, 2 * k2:2 * k2 + 2, nsl],
                        start=False, stop=False, perf_mode=DR,
                    )

            def emit_h(n, hT8, gact):
                ps = psum_of.pop(n)
                nsl = slice(n * 512, (n + 1) * 512)
                for k2 in range(KH // 2):
                    nc.tensor.matmul(
                        ps[:], hT8[:, 2 * k2:2 * k2 + 2, :],
                        whh[:, 2 * k2:2 * k2 + 2, nsl],
                        start=False, stop=(k2 == KH // 2 - 1), perf_mode=DR,
                    )
                nc.scalar.activation(gact[:, nsl], ps[:], _BANK_FUNC[n],
                                     scale=1.0 / GS)

            def emit_fc_chunk(t, hT, v):
                fp = pfpool.tile([B, VC], F32, tag="pf")
                for k in range(KH):
                    nc.tensor.matmul(
                        fp[:], hT[:, k, :], wfc[:, k, v * VC:(v + 1) * VC],
                        start=(k == 0), stop=(k == KH - 1),
                    )
                lo = lpool.tile([B, VC], F32, tag="lo")
                nc.vector.tensor_add(lo[:], fp[:], bfc[:, v * VC:(v + 1) * VC])
                nc.scalar.dma_start(out_d[t, :, v * VC:(v + 1) * VC], lo[:])

            def emit_transposes(h_bf):
                hT = hpool.tile([128, KH, B], BF16, tag="hT")
                hT8 = hpool.tile([128, KH, B], F8, tag="hT8")
                for ch in range(KH):
                    pt = ptpool.tile([128, B], BF16, tag="pt")
                    nc.tensor.transpose(
                        pt[:], h_bf[:, ch * 128:(ch + 1) * 128], iden[:])
                    nc.vector.tensor_copy(hT[:, ch, :], pt[:])
                    nc.scalar.activation(hT8[:, ch, :], pt[:], AF.Copy,
                                         scale=XS)
                return hT, hT8

            h_bf_prev = None

            for t in range(1, T):
                xt = xt_all[:, t, :, :]
                bo = bank_order

                # ---- (A0) bias + x-part for banks 0-3 (no h dependency) ----
                for n in bo[:4]:
                    emit_bias_x(n, xt)

                # ---- (B) transposes of h(t-1) -> hT, hT8 ----
                if t > 1:
                    hT_prev, hT8_prev = emit_transposes(h_bf_prev)

                if t == 1:
                    for n in bank_order:
                        nsl = slice(n * 512, (n + 1) * 512)
                        nc.sync.dma_start(whh[:, :, nsl], whh_d[:, :, nsl])

                gact = gpool.tile([B, 4 * H], F32, tag="gact")

                # ---- (C/D interleave): FC(t-1), h-DR(t), bias+x rest ----
                emit_fc_chunk(t - 1, hT_prev, 0)
                emit_h(bo[0], hT8_prev, gact)
                emit_h(bo[1], hT8_prev, gact)
                emit_bias_x(bo[4], xt)
                emit_h(bo[2], hT8_prev, gact)
                emit_bias_x(bo[5], xt)
                emit_h(bo[3], hT8_prev, gact)
                emit_bias_x(bo[6], xt)
                emit_fc_chunk(t - 1, hT_prev, 1)
                emit_h(bo[4], hT8_prev, gact)
                emit_bias_x(bo[7], xt)
                emit_h(bo[5], hT8_prev, gact)
                emit_fc_chunk(t - 1, hT_prev, 2)
                emit_h(bo[6], hT8_prev, gact)
                emit_h(bo[7], hT8_prev, gact)

                i_g = gact[:, 0:H]
                f_g = gact[:, H:2 * H]
                g_g = gact[:, 2 * H:3 * H]
                o_g = gact[:, 3 * H:4 * H]

                # ---- (E) c, h tail in halves ----
                ig = tpool.tile([B, H], F32, tag="ig")
                tanh_c = tpool.tile([B, H], F32, tag="tanh_c")
                h_bf = hpool.tile([B, H], BF16, tag="h_bf")
                HHH = H // 2
                for half in range(2):
                    hsl = slice(half * HHH, (half + 1) * HHH)
                    nc.vector.tensor_mul(ig[:, hsl], i_g[:, hsl], g_g[:, hsl])
                    nc.vector.tensor_mul(c[:, hsl], c[:, hsl], f_g[:, hsl])
                    nc.vector.tensor_add(c[:, hsl], c[:, hsl], ig[:, hsl])
                    nc.scalar.activation(tanh_c[:, hsl], c[:, hsl], AF.Tanh)
                    nc.vector.tensor_mul(h_bf[:, hsl], o_g[:, hsl],
                                         tanh_c[:, hsl])
                h_bf_prev = h_bf

            # epilogue: transpose h(T-1) halves interleaved with FC(T-1)
            hT_last = hpool.tile([128, KH, B], BF16, tag="hT")
            for ch in range(KH // 2):
                pt = ptpool.tile([128, B], BF16, tag="pt")
                nc.tensor.transpose(
                    pt[:], h_bf_prev[:, ch * 128:(ch + 1) * 128], iden[:])
                nc.vector.tensor_copy(hT_last[:, ch, :], pt[:])
            fp0 = pfpool.tile([B, VC], F32, tag="pf", name="fpe0")
            fp1 = pfpool.tile([B, VC], F32, tag="pf", name="fpe1")
            for v, fp in ((0, fp0), (1, fp1)):
                for k in range(KH // 2):
                    nc.tensor.matmul(
                        fp[:], hT_last[:, k, :],
                        wfc[:, k, v * VC:(v + 1) * VC],
                        start=(k == 0), stop=False)
            for ch in range(KH // 2, KH):
                pt = ptpool.tile([128, B], BF16, tag="pt")
                nc.tensor.transpose(
                    pt[:], h_bf_prev[:, ch * 128:(ch + 1) * 128], iden[:])
                nc.vector.tensor_copy(hT_last[:, ch, :], pt[:])
            for v, fp in ((0, fp0), (1, fp1)):
                for k in range(KH // 2, KH):
                    nc.tensor.matmul(
                        fp[:], hT_last[:, k, :],
                        wfc[:, k, v * VC:(v + 1) * VC],
                        start=False, stop=(k == KH - 1))
                lo = lpool.tile([B, VC], F32, tag="lo")
                nc.vector.tensor_add(lo[:], fp[:],
                                     bfc[:, v * VC:(v + 1) * VC])
                nc.scalar.dma_start(out_d[T - 1, :, v * VC:(v + 1) * VC], lo[:])
            fp2 = pfpool.tile([B, VC], F32, tag="pf", name="fpe2")
            for k in range(KH):
                nc.tensor.matmul(
                    fp2[:], hT_last[:, k, :], wfc[:, k, 2 * VC:3 * VC],
                    start=(k == 0), stop=(k == KH - 1))
            lo2 = lpool.tile([B, VC], F32, tag="lo")
            nc.vector.tensor_add(lo2[:], fp2[:], bfc[:, 2 * VC:3 * VC])
            nc.scalar.dma_start(out_d[T - 1, :, 2 * VC:2 * VC + VC // 2],
                                lo2[:, 0:VC // 2])
            nc.sync.dma_start(out_d[T - 1, :, 2 * VC + VC // 2:3 * VC],
                              lo2[:, VC // 2:VC])

    _split_multi_waits(nc)
    return nc


_NC_CACHE = None


def _get_nc():
    global _NC_CACHE
    if _NC_CACHE is None:
        _NC_CACHE = build_nc()
    return _NC_CACHE


def _prep_inputs(encoder_output, captions, embed_table, W_ih, W_hh, b_ih, b_hh,
                 W_fc, b_fc):
    bf = ml_dtypes.bfloat16
    f8 = ml_dtypes.float8_e4m3fn
    enc = np.asarray(encoder_output, np.float32)
    cap = np.asarray(captions).astype(np.int64)
    emb = np.asarray(embed_table, np.float32)
    W_ih = np.asarray(W_ih, np.float32)
    W_hh = np.asarray(W_hh, np.float32)
    W_fc = np.asarray(W_fc, np.float32)
    bg = (np.asarray(b_ih, np.float32) + np.asarray(b_hh, np.float32))
    b_fc = np.asarray(b_fc, np.float32)

    X = np.empty((T, B, E), np.float32)
    X[0] = enc
    X[1:] = emb[cap[:, : T - 1]].transpose(1, 0, 2)
    # xt[t,p,k,b] = X[t,b,k*128+p] * XS  (fp8)
    xt = np.ascontiguousarray(
        (X * XS).reshape(T, B, KX, 128).transpose(0, 3, 2, 1)).astype(f8)

    # step 0 on host, fp32 (h_prev = c_prev = 0)
    gates0 = enc @ W_ih.T + bg
    i0, f0, g0, o0 = np.split(gates0, 4, axis=-1)
    sig = lambda z: 1.0 / (1.0 + np.exp(-z))
    c0 = sig(i0) * np.tanh(g0)
    h0 = sig(o0) * np.tanh(c0)
    h0_bf = h0.astype(bf).astype(np.float32)
    ht0 = np.ascontiguousarray(
        h0_bf.T.reshape(KH, 128, B).transpose(1, 0, 2)).astype(bf)
    ht08 = np.ascontiguousarray(
        (h0_bf.T * XS).reshape(KH, 128, B).transpose(1, 0, 2)).astype(f8)
    wih = np.ascontiguousarray(
        (W_ih.T * WS).reshape(KX, 128, 4 * H).transpose(1, 0, 2)).astype(f8)
    whh = np.ascontiguousarray(
        (W_hh.T * WS).reshape(KH, 128, 4 * H).transpose(1, 0, 2)).astype(f8)
    bg_row = np.ascontiguousarray((bg * GS)[None, :]).astype(bf)
    one_row = np.zeros((128, B), bf)
    one_row[0] = 1.0
    iden = np.eye(128, dtype=bf)

    common = {"wih": wih, "whh": whh, "xt": xt, "bg": bg_row, "one": one_row,
              "iden": iden, "ht0": ht0, "ht08": ht08,
              "c0": np.ascontiguousarray(c0, np.float32)}
    in_maps = []
    for ci in range(NCORES):
        sl = slice(ci * VS, (ci + 1) * VS)
        wfc = np.ascontiguousarray(
            W_fc[sl].reshape(VS, KH, 128).transpose(2, 1, 0)).astype(bf)
        bfc = np.ascontiguousarray(np.broadcast_to(b_fc[sl], (128, VS)))
        in_maps.append({**common, "wfc": wfc, "bfc": bfc})
    return in_maps


def run_on_device(in_maps, trace=False, **kw):
    nc = _get_nc()
    return run_bass_kernel_spmd(
        nc, in_maps, list(range(NCORES)), trace=trace, **kw)


def kernel(encoder_output, captions, embed_table, W_ih, W_hh, b_ih, b_hh,
           W_fc, b_fc):
    in_maps = _prep_inputs(encoder_output, captions, embed_table,
                           W_ih, W_hh, b_ih, b_hh, W_fc, b_fc)
    res = run_on_device(in_maps)
    shards = [np.asarray(res.results[ci]["logits"]) for ci in range(NCORES)]
    full = np.concatenate(shards, axis=-1)  # [T, B, V]
    return np.ascontiguousarray(full.transpose(1, 0, 2))  # [B, T, V]


# revision 5
# speedup vs baseline: 1.0424x; 1.0104x over previous
"""LSTM caption decoder on 8 TRN2 NeuronCores — fp8 DoubleRow gates.

Problem: 24-step LSTMCell (B=128, E=512, H=1024) + vocab projection (V=12000).

Strategy (no collectives), evolved from the bf16 baseline:
  - Every core computes the full-batch LSTM redundantly; FC vocab projection
    sharded column-wise 8 ways (1500 cols/core), interleaved into the loop.
  - Gate matmuls run in fp8 e4m3 with MatmulPerfMode.DoubleRow: one
    instruction contracts TWO 128-row K-chunks (measured 2x bf16 MACs/s).
    Scaling keeps everything in e4m3's normal range: x*64, h*64, W*16;
    PSUM holds 1024*gates; ACT applies 1/1024 with the activation.
  - Gate bias folded into the matmul as a K=128 zero-padded ones-column
    (a K=1 row costs ~450ns per K-size transition next to K=256 DR matmuls;
    K=128<->256 transitions are free). Only row 0 of the bias operand is
    real: the tile is memset once and an 8KB row-0 DMA fills it.
  - h -> hT via PE transposes (identity matmul) instead of DMA transpose:
    the 6us element-strided DMA transpose was the per-step critical path and
    its PE idle window dropped the clock to the mid p-state (427ns matmuls).
    The transpose PSUM tile feeds BOTH the bf16 hT (DVE copy, for FC) and
    the fp8 hT8 (ACT Copy scale=64, for the recurrence).
  - PE stream is software-pipelined so it never idles: per segment
    [bias+x(t) banks 0-3] [transposes(t-1) halves around FC(t-1) v0
    k-halves] [h-DR(t) + bias+x(t) rest + FC(t-1) v1/v2 interleaved] —
    bias/x have no recurrent dependency and cover the h(t-1) tail latency.
    Segment 1 front-loads all of FC_0 against the whh DMA arrival; the
    last segment's FC v2 fills the final tail gap. PSUM: pg=4 gate banks +
    pf=2 FC + pt=2 transpose tiles = 8 banks; a bias+x allocation may only
    be emitted after the 4-older bank's ACT drain (ring dependency).

End-to-end rel err vs fp32 reference: 1.43e-2 (measured on HW; matches the
numpy sim of this exact scheme; W-quantization dominates).
"""

import sys

if "/opt/trn_rl_repo" not in sys.path:
    sys.path.insert(0, "/opt/trn_rl_repo")

import numpy as np
import ml_dtypes

import concourse.bass as bass
import concourse.tile as tile
from concourse import mybir
from concourse.bass_utils import run_bass_kernel_spmd

B = 128
T = 24
E = 512
H = 1024
V = 12000
NCORES = 8
VS = V // NCORES          # 1500 vocab cols per core
KX = E // 128             # 4 contraction chunks for x-part
KH = H // 128             # 8 contraction chunks for h-part
NG = (4 * H) // 512       # 8 gate psum banks of 512
NVC = 3                   # vocab chunks per core (3 x 500)
VC = VS // NVC            # 500

XS = 64.0                 # fp8 scale for x and h
WS = 16.0                 # fp8 scale for W_ih / W_hh
GS = XS * WS              # psum carries GS * gates

F32 = mybir.dt.float32
BF16 = mybir.dt.bfloat16
F8 = mybir.dt.float8e4
AF = mybir.ActivationFunctionType
DR = mybir.MatmulPerfMode.DoubleRow

# gate bank n -> activation (torch order i,f,g,o over 4H)
_BANK_FUNC = [AF.Sigmoid] * 4 + [AF.Tanh] * 2 + [AF.Sigmoid] * 2


def _split_multi_waits(nc) -> int:
    """Walrus here allows exactly one sync-wait per 64B instruction (one
    NEURON_ISA_TPB_EVENTS slot). Tile attaches all outstanding waits to one
    instruction; rewrite  inst[wA,wB,wC] -> nop[wA]; nop[wB]; inst[wC]."""
    n = 0
    for fn in nc.m.functions:
        for bb in fn.blocks:
            insts = bb.instructions
            out = []
            changed = False
            for inst in insts:
                si = getattr(inst, "sync_info", None)
                waits = list(si.on_wait) if si is not None and si.on_wait else []
                if len(waits) > 1:
                    changed = True
                    for w in waits[:-1]:
                        nop = mybir.InstNoOp(
                            name=nc.get_next_instruction_name(),
                            sync_info=mybir.SyncInfo(on_wait=[w], on_update=[]),
                            bass_nofuse=True,
                            engine=inst.engine,
                        )
                        nc.register_instruction(nop, overwrite=True)
                        out.append(nop)
                        n += 1
                    inst.sync_info = mybir.SyncInfo(
                        on_wait=[waits[-1]], on_update=list(si.on_update or [])
                    )
                out.append(inst)
            if changed:
                insts.clear()
                insts.extend(out)
    return n


def build_nc():
    nc = bass.Bass("TRN2", target_bir_lowering=False, debug=False, num_devices=NCORES)

    wih_d = nc.dram_tensor("wih", [128, KX, 4 * H], F8, kind="ExternalInput").ap()
    whh_d = nc.dram_tensor("whh", [128, KH, 4 * H], F8, kind="ExternalInput").ap()
    wfc_d = nc.dram_tensor("wfc", [128, KH, VS], BF16, kind="ExternalInput").ap()
    xt_d = nc.dram_tensor("xt", [T, 128, KX, B], F8, kind="ExternalInput").ap()
    bg_d = nc.dram_tensor("bg", [1, 4 * H], BF16, kind="ExternalInput").ap()
    bfc_d = nc.dram_tensor("bfc", [128, VS], F32, kind="ExternalInput").ap()
    ht0_d = nc.dram_tensor("ht0", [128, KH, B], BF16, kind="ExternalInput").ap()
    ht08_d = nc.dram_tensor("ht08", [128, KH, B], F8, kind="ExternalInput").ap()
    c0_d = nc.dram_tensor("c0", [B, H], F32, kind="ExternalInput").ap()
    one_d = nc.dram_tensor("one", [128, B], BF16, kind="ExternalInput").ap()
    iden_d = nc.dram_tensor("iden", [128, 128], BF16, kind="ExternalInput").ap()
    out_d = nc.dram_tensor("logits", [T, B, VS], F32, kind="ExternalOutput").ap()

    with tile.TileContext(nc) as tc:
        with (
            tc.tile_pool(name="weights", bufs=1) as wpool,
            tc.tile_pool(name="xin", bufs=1) as xpool,
            tc.tile_pool(name="gact", bufs=2) as gpool,
            tc.tile_pool(name="state", bufs=1) as spool,
            tc.tile_pool(name="tmp", bufs=1) as tpool,
            tc.tile_pool(name="hbuf", bufs=2) as hpool,
            tc.tile_pool(name="lout", bufs=4) as lpool,
            tc.tile_pool(name="pg", bufs=4, space="PSUM") as pgpool,
            tc.tile_pool(name="pf", bufs=2, space="PSUM") as pfpool,
            tc.tile_pool(name="pt", bufs=2, space="PSUM") as ptpool,
        ):
            # Prologue loads, consumer-ordered so early PE work starts asap.
            wih = wpool.tile([128, KX, 4 * H], F8)
            whh = wpool.tile([128, KH, 4 * H], F8)
            bg = wpool.tile([128, 4 * H], BF16)
            one = wpool.tile([128, B], BF16)
            iden = wpool.tile([128, 128], BF16)
            wfc = wpool.tile([128, KH, VS], BF16)
            bfc = wpool.tile([128, VS], F32)
            ht0 = hpool.tile([128, KH, B], BF16, tag="hT")
            ht08 = hpool.tile([128, KH, B], F8, tag="hT8")
            nc.sync.dma_start(one[:], one_d[:])
            nc.sync.dma_start(bg[:], bg_d[:])
            nc.sync.dma_start(iden[:], iden_d[:])
            nc.scalar.dma_start(ht0[:], ht0_d[:])
            nc.scalar.dma_start(ht08[:], ht08_d[:])
            c = spool.tile([B, H], F32)
            nc.scalar.dma_start(c[:], c0_d[:])
            # wih first (bias+x matmuls are the earliest PE work), then wfc
            # (FC_0), then whh in bank order.
            for n in range(NG):
                nsl = slice(n * 512, (n + 1) * 512)
                nc.sync.dma_start(wih[:, :, nsl], wih_d[:, :, nsl])
            xt_all = xpool.tile([128, T, KX, B], F8)
            for tt in range(1, T):
                nc.sync.dma_start(xt_all[:, tt, :, :], xt_d[tt])
            for v in range(NVC):
                vsl = slice(v * VC, (v + 1) * VC)
                nc.scalar.dma_start(wfc[:, :, vsl], wfc_d[:, :, vsl])
            nc.scalar.dma_start(bfc[:], bfc_d[:])

            hT_prev = ht0
            hT8_prev = ht08

            # gate-bank order: i(0,1), g(4,5), f(2,3), o(6,7) so the
            # c-chain (needs i,g then f) starts before o's sigmoid lands
            bank_order = [0, 1, 4, 5, 2, 3, 6, 7]

            psum_of = {}      # bank -> live psum tile (bias+x accumulated)

            def emit_bias_x(n, xt):
                ps = pgpool.tile([B, 512], F32, tag="pg")
                psum_of[n] = ps
                nsl = slice(n * 512, (n + 1) * 512)
                nc.tensor.matmul(ps[:], one[:], bg[:, nsl],
                                 start=True, stop=False)
                for k2 in range(KX // 2):
                    nc.tensor.matmul(
                        ps[:], xt[:, 2 * k2:2 * k2 + 2, :],
                        wih[:, 2 * k2:2 * k2 + 2, nsl],
                        start=False, stop=False, perf_mode=DR,
                    )

            def emit_h(n, hT8, gact):
                ps = psum_of.pop(n)
                nsl = slice(n * 512, (n + 1) * 512)
                for k2 in range(KH // 2):
                    nc.tensor.matmul(
                        ps[:], hT8[:, 2 * k2:2 * k2 + 2, :],
                        whh[:, 2 * k2:2 * k2 + 2, nsl],
                        start=False, stop=(k2 == KH // 2 - 1), perf_mode=DR,
                    )
                nc.scalar.activation(gact[:, nsl], ps[:], _BANK_FUNC[n],
                                     scale=1.0 / GS)

            def emit_fc_chunk(t, hT, v):
                fp = pfpool.tile([B, VC], F32, tag="pf")
                for k in range(KH):
                    nc.tensor.matmul(
                        fp[:], hT[:, k, :], wfc[:, k, v * VC:(v + 1) * VC],
                        start=(k == 0), stop=(k == KH - 1),
                    )
                lo = lpool.tile([B, VC], F32, tag="lo")
                nc.vector.tensor_add(lo[:], fp[:], bfc[:, v * VC:(v + 1) * VC])
                nc.scalar.dma_start(out_d[t, :, v * VC:(v + 1) * VC], lo[:])

            def emit_transposes(h_bf):
                hT = hpool.tile([128, KH, B], BF16, tag="hT")
                hT8 = hpool.tile([128, KH, B], F8, tag="hT8")
                for ch in range(KH):
                    pt = ptpool.tile([128, B], BF16, tag="pt")
                    nc.tensor.transpose(
                        pt[:], h_bf[:, ch * 128:(ch + 1) * 128], iden[:])
                    nc.vector.tensor_copy(hT[:, ch, :], pt[:])
                    nc.scalar.activation(hT8[:, ch, :], pt[:], AF.Copy,
                                         scale=XS)
                return hT, hT8

            h_bf_prev = None

            for t in range(1, T):
                xt = xt_all[:, t, :, :]
                bo = bank_order

                # ---- (A0) bias + x-part for banks 0-3 (no h dependency) ----
                for n in bo[:4]:
                    emit_bias_x(n, xt)

                # ---- (B) transposes of h(t-1) -> hT, hT8 ----
                if t > 1:
                    hT_prev, hT8_prev = emit_transposes(h_bf_prev)

                if t == 1:
                    for n in bank_order:
                        nsl = slice(n * 512, (n + 1) * 512)
                        nc.sync.dma_start(whh[:, :, nsl], whh_d[:, :, nsl])

                gact = gpool.tile([B, 4 * H], F32, tag="gact")

                # ---- (C/D interleave): FC(t-1), h-DR(t), bias+x rest ----
                emit_fc_chunk(t - 1, hT_prev, 0)
                emit_h(bo[0], hT8_prev, gact)
                emit_h(bo[1], hT8_prev, gact)
                emit_bias_x(bo[4], xt)
                emit_h(bo[2], hT8_prev, gact)
                emit_bias_x(bo[5], xt)
                emit_h(bo[3], hT8_prev, gact)
                emit_bias_x(bo[6], xt)
                emit_fc_chunk(t - 1, hT_prev, 1)
                emit_h(bo[4], hT8_prev, gact)
                emit_bias_x(bo[7], xt)
                emit_h(bo[5], hT8_prev, gact)
                emit_fc_chunk(t - 1, hT_prev, 2)
                emit_h(bo[6], hT8_prev, gact)
                emit_h(bo[7], hT8_prev, gact)

                i_g = gact[:, 0:H]
                f_g = gact[:, H:2 * H]
                g_g = gact[:, 2 * H:3 * H]
                o_g = gact[:, 3 * H:4 * H]

                # ---- (E) c, h tail in halves ----
                ig = tpool.tile([B, H], F32, tag="ig")
                tanh_c = tpool.tile([B, H], F32, tag="tanh_c")
                h_bf = hpool.tile([B, H], BF16, tag="h_bf")
                HHH = H // 2
                for half in range(2):
                    hsl = slice(half * HHH, (half + 1) * HHH)
                    nc.vector.tensor_mul(ig[:, hsl], i_g[:, hsl], g_g[:, hsl])
                    nc.vector.tensor_mul(c[:, hsl], c[:, hsl], f_g[:, hsl])
                    nc.vector.tensor_add(c[:, hsl], c[:, hsl], ig[:, hsl])
                    nc.scalar.activation(tanh_c[:, hsl], c[:, hsl], AF.Tanh)
                    nc.vector.tensor_mul(h_bf[:, hsl], o_g[:, hsl],
                                         tanh_c[:, hsl])
                h_bf_prev = h_bf

            # epilogue: transpose h(T-1) halves interleaved with FC(T-1)
            hT_last = hpool.tile([128, KH, B], BF16, tag="hT")
            for ch in range(KH // 2):
                pt = ptpool.tile([128, B], BF16, tag="pt")
                nc.tensor.transpose(
                    pt[:], h_bf_prev[:, ch * 128:(ch + 1) * 128], iden[:])
                nc.vector.tensor_copy(hT_last[:, ch, :], pt[:])
            fp0 = pfpool.tile([B, VC], F32, tag="pf", name="fpe0")
            fp1 = pfpool.tile([B, VC], F32, tag="pf", name="fpe1")
            for v, fp in ((0, fp0), (1, fp1)):
                for k in range(KH // 2):
                    nc.tensor.matmul(
                        fp[:], hT_last[:, k, :],
                        wfc[:, k, v * VC:(v + 1) * VC],
                        start=(k == 0), stop=False)
            for ch in range(KH // 2, KH):
                pt = ptpool.tile([128, B], BF16, tag="pt")
                nc.tensor.transpose(
                    pt[:], h_bf_prev[:, ch * 128:(ch + 1) * 128], iden[:])
                nc.vector.tensor_copy(hT_last[:, ch, :], pt[:])
            for v, fp in ((0, fp0), (1, fp1)):
                for k in range(KH // 2, KH):
                    nc.tensor.matmul(
                        fp[:], hT_last[:, k, :],
                        wfc[:, k, v * VC:(v + 1) * VC],
                        start=False, stop=(k == KH - 1))
                lo = lpool.tile([B, VC], F32, tag="lo")
                nc.vector.tensor_add(lo[:], fp[:],
                                     bfc[:, v * VC:(v + 1) * VC])
                nc.scalar.dma_start(out_d[T - 1, :, v * VC:(v + 1) * VC], lo[:])
            fp2 = pfpool.tile([B, VC], F32, tag="pf", name="fpe2")
            for k in range(KH):
                nc.tensor.matmul(
                    fp2[:], hT_last[:, k, :], wfc[:, k, 2 * VC:3 * VC],
                    start=(k == 0), stop=(k == KH - 1))
            lo2 = lpool.tile([B, VC], F32, tag="lo")
            nc.vector.tensor_add(lo2[:], fp2[:], bfc[:, 2 * VC:3 * VC])
            nc.scalar.dma_start(out_d[T - 1, :, 2 * VC:2 * VC + VC // 2],
                                lo2[:, 0:VC // 2])
            nc.sync.dma_start(out_d[T - 1, :, 2 * VC + VC // 2:3 * VC],
                              lo2[:, VC // 2:VC])

    _split_multi_waits(nc)
    return nc


_NC_CACHE = None


def _get_nc():
    global _NC_CACHE
    if _NC_CACHE is None:
        _NC_CACHE = build_nc()
    return _NC_CACHE


def _prep_inputs(encoder_output, captions, embed_table, W_ih, W_hh, b_ih, b_hh,
                 W_fc, b_fc):
    bf = ml_dtypes.bfloat16
    f8 = ml_dtypes.float8_e4m3fn
    enc = np.asarray(encoder_output, np.float32)
    cap = np.asarray(captions).astype(np.int64)
    emb = np.asarray(embed_table, np.float32)
    W_ih = np.asarray(W_ih, np.float32)
    W_hh = np.asarray(W_hh, np.float32)
    W_fc = np.asarray(W_fc, np.float32)
    bg = (np.asarray(b_ih, np.float32) + np.asarray(b_hh, np.float32))
    b_fc = np.asarray(b_fc, np.float32)

    X = np.empty((T, B, E), np.float32)
    X[0] = enc
    X[1:] = emb[cap[:, : T - 1]].transpose(1, 0, 2)
    # xt[t,p,k,b] = X[t,b,k*128+p] * XS  (fp8)
    xt = np.ascontiguousarray(
        (X * XS).reshape(T, B, KX, 128).transpose(0, 3, 2, 1)).astype(f8)

    # step 0 on host, fp32 (h_prev = c_prev = 0)
    gates0 = enc @ W_ih.T + bg
    i0, f0, g0, o0 = np.split(gates0, 4, axis=-1)
    sig = lambda z: 1.0 / (1.0 + np.exp(-z))
    c0 = sig(i0) * np.tanh(g0)
    h0 = sig(o0) * np.tanh(c0)
    h0_bf = h0.astype(bf).astype(np.float32)
    ht0 = np.ascontiguousarray(
        h0_bf.T.reshape(KH, 128, B).transpose(1, 0, 2)).astype(bf)
    ht08 = np.ascontiguousarray(
        (h0_bf.T * XS).reshape(KH, 128, B).transpose(1, 0, 2)).astype(f8)
    wih = np.ascontiguousarray(
        (W_ih.T * WS).reshape(KX, 128, 4 * H).transpose(1, 0, 2)).astype(f8)
    whh = np.ascontiguousarray(
        (W_hh.T * WS).reshape(KH, 128, 4 * H).transpose(1, 0, 2)).astype(f8)
    bg_row = np.ascontiguousarray((bg * GS)[None, :]).astype(bf)
    one_row = np.zeros((128, B), bf)
    one_row[0] = 1.0
    iden = np.eye(128, dtype=bf)

    common = {"wih": wih, "whh": whh, "xt": xt, "bg": bg_row, "one": one_row,
              "iden": iden, "ht0": ht0, "ht08": ht08,
              "c0": np.ascontiguousarray(c0, np.float32)}
    in_maps = []
    for ci in range(NCORES):
        sl = slice(ci * VS, (ci + 1) * VS)
        wfc = np.ascontiguousarray(
            W_fc[sl].reshape(VS, KH, 128).transpose(2, 1, 0)).astype(bf)
        bfc = np.ascontiguousarray(np.broadcast_to(b_fc[sl], (128, VS)))
        in_maps.append({**common, "wfc": wfc, "bfc": bfc})
    return in_maps


def run_on_device(in_maps, trace=False, **kw):
    nc = _get_nc()
    return run_bass_kernel_spmd(
        nc, in_maps, list(range(NCORES)), trace=trace, **kw)


def kernel(encoder_output, captions, embed_table, W_ih, W_hh, b_ih, b_hh,
           W_fc, b_fc):
    in_maps = _prep_inputs(encoder_output, captions, embed_table,
                           W_ih, W_hh, b_ih, b_hh, W_fc, b_fc)
    res = run_on_device(in_maps)
    shards = [np.asarray(res.results[ci]["logits"]) for ci in range(NCORES)]
    full = np.concatenate(shards, axis=-1)  # [T, B, V]
    return np.ascontiguousarray(full.transpose(1, 0, 2))  # [B, T, V]
